# revision 14
# baseline (speedup 1.0000x reference)
"""Trainium2 Bass kernel for a pre-norm transformer block (B=1, T=4096, C=1024, H=16).

Sharding (8 cores): head-tensor-parallel attention (2 heads/core) with
sequence-parallel LayerNorm statistics and sequence-local MLP.
Activations are kept transposed on-chip ([C, T] with C on partitions)
so every matmul contracts over the partition axis with K=128 chunks.

Per core c (owns T-columns [512c, 512c+512) and heads 2c, 2c+1):
  1. LN1 statistics on own xT columns -> mu, rstd [1, 512]
  2. Tiny AllGather of (rstd, mu*rstd) [1,1024] bf16 -> [8,1024].
     LN is affine in x, so  qkv = rstd*(x @ W') - (mu*rstd)*colsum(W')
     with W' = diag(ln1_w) @ w_qkv folded host-side: QKV for the full
     sequence is computed locally from a prefetched bf16 copy of x —
     no big h1 AllGather.
  3. causal flash attention, no max subtraction (scores are O(1)):
     scores for 2 k-tiles land in a triple-buffered 2-bank PSUM tile,
     one batched exp per pair (ACT pipelines against PE), PV with
     V-stationary [128,65] (65th col of ones accumulates the softmax
     denominator l) -> unnormalized o plus l per query
  4. AllToAll (bf16) ships unnormalized o + l; receiver computes
     1/l for all 16 heads at once (ACT ln/exp), broadcasts over the
     64 head dims with a tiny matmul, scales, then
     proj: x2T = xT + wproj.T @ aoutT
  5. LN2 -> h2T (bf16); MLP with bf16 weights; out = x2T + mlpT

All matmul operands are bf16 (f32r moving streams at half rate).
"""
import numpy as np
import ml_dtypes

import concourse.bass as bass
import concourse.bacc as bacc
import concourse.tile as tile
import concourse.mybir as mybir
from concourse import bass_utils

F32 = mybir.dt.float32
F32R = mybir.dt.float32r
BF16 = mybir.dt.bfloat16
AF = mybir.ActivationFunctionType
OP = mybir.AluOpType

NCORES = 8
C = 1024
T = 4096
TC = T // NCORES          # 512 own T columns
CK = C // 128             # 8 C chunks
HS = 64
FC = 4096                 # MLP hidden
EPS = 1e-5

_CACHE = {}
DEBUG = False


def _layer_norm(nc, tc, x_t, w_ap, out_t, eps_t, ones_t, ones_b, tag):
    """Full LN producing out_t [128, CK, 512] BF16 (used for LN2)."""
    with tc.tile_pool(name=f"ln{tag}", bufs=1) as lnp, \
         tc.tile_pool(name=f"lnps{tag}", bufs=1, space="PSUM") as lps:
        mean_ps = lps.tile([128, 512], F32, name=f"mean{tag}")
        sq_ps = lps.tile([128, 512], F32, name=f"sqs{tag}")
        sq_tiles = []
        for k in range(CK):
            sq = lnp.tile([128, 512], BF16, name=f"sq{tag}", bufs=2)
            xk = x_t[:, k, :].bitcast(F32)
            nc.vector.tensor_mul(sq[:], xk, xk)
            sq_tiles.append(sq)
        for k in range(CK):
            nc.tensor.matmul(mean_ps[:], ones_t[:], x_t[:, k, :],
                             start=(k == 0), stop=(k == CK - 1))
        for k in range(CK):
            nc.tensor.matmul(sq_ps[:], ones_b[:], sq_tiles[k][:],
                             start=(k == 0), stop=(k == CK - 1))
        mu = lnp.tile([128, 512], F32, name=f"mu{tag}")
        nc.vector.tensor_scalar_mul(mu[:], mean_ps[:], 1.0 / C)
        musq = lnp.tile([128, 512], F32, name=f"musq{tag}")
        nc.vector.tensor_mul(musq[:], mu[:], mu[:])
        var = lnp.tile([128, 512], F32, name=f"var{tag}")
        nc.vector.scalar_tensor_tensor(var[:], sq_ps[:], 1.0 / C, musq[:],
                                       OP.mult, OP.subtract)
        lnv = lnp.tile([128, 512], F32, name=f"lnv{tag}")
        nc.scalar.activation(lnv[:], var[:], AF.Ln, bias=eps_t[:])
        rstd = lnp.tile([128, 512], F32, name=f"rstd{tag}")
        nc.scalar.activation(rstd[:], lnv[:], AF.Exp, scale=-0.5)
        for k in range(CK):
            d = lnp.tile([128, 512], F32, name=f"d{tag}", bufs=2)
            nc.vector.tensor_sub(d[:], x_t[:, k, :].bitcast(F32), mu[:])
            nc.vector.scalar_tensor_tensor(
                out_t[:, k, :], d[:], w_ap[:, k:k + 1],
                rstd[:], OP.mult, OP.mult)


def _build():
    nc = bacc.Bacc("TRN2", target_bir_lowering=False, debug=False,
                   enable_asserts=False, num_devices=NCORES)

    xT = nc.dram_tensor("xT", [C, TC], F32, kind="ExternalInput").ap()
    xb = nc.dram_tensor("xb", [128, CK, T], BF16, kind="ExternalInput").ap()
    wqkv = nc.dram_tensor("wqkv", [128, CK, 3 * 128], BF16,
                          kind="ExternalInput").ap()
    wqsn = nc.dram_tensor("wqsn", [1, 3 * 128], BF16, kind="ExternalInput").ap()
    wproj = nc.dram_tensor("wproj", [128, CK, C], BF16,
                           kind="ExternalInput").ap()
    wfc = nc.dram_tensor("wfc", [128, FC // 128, CK, 128], BF16,
                         kind="ExternalInput").ap()
    wmlp = nc.dram_tensor("wmlp", [128, FC // 128, C], BF16,
                          kind="ExternalInput").ap()
    ln2w = nc.dram_tensor("ln2w", [128, CK], F32, kind="ExternalInput").ap()
    masks = nc.dram_tensor("masks", [128, 4 * 512], BF16,
                           kind="ExternalInput").ap()
    ident = nc.dram_tensor("ident", [128, 128], BF16, kind="ExternalInput").ap()
    onesw = nc.dram_tensor("onesw", [128, 128], F32, kind="ExternalInput").ap()
    sel16 = nc.dram_tensor("sel16", [16, CK * 128], BF16,
                           kind="ExternalInput").ap()
    zeros = nc.dram_tensor("zeros", [64, T], BF16, kind="ExternalInput").ap()
    outT = nc.dram_tensor("outT", [C, TC], F32, kind="ExternalOutput").ap()
    dbg = {}
    if DEBUG:
        for nm in ("d_qp0", "d_qp1", "d_kT", "d_vT"):
            dbg[nm] = nc.dram_tensor(nm, [128, T], BF16,
                                     kind="ExternalOutput").ap()
        dbg["d_mrall"] = nc.dram_tensor("d_mrall", [8, 1024], BF16,
                                        kind="ExternalOutput").ap()
        dbg["d_oTe0"] = nc.dram_tensor("d_oTe0", [65, T], BF16,
                                       kind="ExternalOutput").ap()
        dbg["d_oTe1"] = nc.dram_tensor("d_oTe1", [65, T], BF16,
                                       kind="ExternalOutput").ap()
        dbg["d_x2T"] = nc.dram_tensor("d_x2T", [128, T], F32,
                                      kind="ExternalOutput").ap()

    rg = [list(range(NCORES))]

    with tile.TileContext(nc) as tc:
        with tc.tile_pool(name="dram", bufs=1, space="DRAM") as dramp:
            # AllToAll payload: per 130-row block: 128 unnormalized o rows
            # (head a dims 0:64, head b dims 64:128), then 2 l rows.
            ob = dramp.tile([NCORES * 130, TC], BF16, name="ob")
            oax = dramp.tile([NCORES * 130, TC], BF16, name="oax")

            with tc.tile_pool(name="glob", bufs=1) as gp:
                ident_t = gp.tile([128, 128], BF16, name="ident_t")
                nc.sync.dma_start(ident_t[:], ident[:])
                ones_t = gp.tile([128, 128], F32R, name="ones_t")
                nc.sync.dma_start(ones_t[:], onesw[:].bitcast(F32R))
                ones_b = gp.tile([128, 128], BF16, name="ones_b")
                nc.vector.memset(ones_b[:], 1.0)
                sel16_t = gp.tile([16, CK * 128], BF16, name="sel16_t")
                nc.sync.dma_start(sel16_t[:], sel16[:])
                ln2w_t = gp.tile([128, CK], F32, name="ln2w_t")
                nc.sync.dma_start(ln2w_t[:], ln2w[:])
                masks_t = gp.tile([128, 4 * 512], BF16, name="masks_t")
                nc.sync.dma_start(masks_t[:], masks[:])
                eps_t = gp.tile([128, 1], F32, name="eps_t")
                nc.vector.memset(eps_t[:], EPS)
                wq_t = gp.tile([128, CK, 3 * 128], BF16, name="wq_t")
                nc.sync.dma_start(wq_t[:], wqkv[:])
                wqsn_t = gp.tile([1, 3 * 128], BF16, name="wqsn_t")
                nc.sync.dma_start(wqsn_t[:], wqsn[:])
                xT_t = gp.tile([128, CK, 512], F32R, name="xT_t")
                nc.sync.dma_start(
                    xT_t[:], xT.rearrange("(k p) c -> p k c", p=128)
                    .bitcast(F32R))
                x2T_t = gp.tile([128, CK, 512], F32R, name="x2T_t")

                # ---------------- attention scope ----------------
                with tc.tile_pool(name="attn", bufs=1) as ap:
                    # persistent attention tensors
                    qp0 = ap.tile([128, T], BF16, name="qp0")
                    qp1 = ap.tile([128, T], BF16, name="qp1")
                    kT_t = ap.tile([128, T], BF16, name="kT_t")
                    vT_t = ap.tile([128, T], BF16, name="vT_t")
                    # unnormalized head outputs + l (row 64), one per head
                    oTe0 = ap.tile([65, T], BF16, name="oTe0")
                    oTe1 = ap.tile([65, T], BF16, name="oTe1")
                    nc.sync.dma_start(qp0[64:128, :], zeros[:])
                    nc.sync.dma_start(qp1[0:64, :], zeros[:])

                    # LN1 stats for ALL tokens, computed locally from bf16 x
                    # (no collective): mean via ones-matmul broadcasts over
                    # partitions for free.  rb = rstd, mbr = mu*rstd row.
                    qdst = [None, kT_t, vT_t]
                    with tc.tile_pool(name="xbp", bufs=1) as xbp, \
                         tc.tile_pool(name="qkvps", bufs=1, space="PSUM") as qps, \
                         tc.tile_pool(name="qbps", bufs=1, space="PSUM") as qbp:
                        xb_t = xbp.tile([128, CK, T], BF16, name="xb_t")
                        for j in range(NCORES):
                            nc.sync.dma_start(
                                xb_t[:, :, 512 * j:512 * (j + 1)],
                                xb[:, :, 512 * j:512 * (j + 1)])
                        rb_t = []
                        mbr_t = []
                        for j in range(NCORES):
                            blk = slice(512 * j, 512 * (j + 1))
                            mean_ps = qbp.tile([128, 512], F32, name="meanj",
                                               bufs=2)
                            sq_ps = qbp.tile([128, 512], F32, name="sqj",
                                             bufs=2)
                            sq_tiles = []
                            for k in range(CK):
                                sq = xbp.tile([128, 512], BF16, name="sqv",
                                              bufs=3)
                                nc.vector.tensor_mul(sq[:], xb_t[:, k, blk],
                                                     xb_t[:, k, blk])
                                sq_tiles.append(sq)
                            for k in range(CK):
                                nc.tensor.matmul(mean_ps[:], ones_b[:],
                                                 xb_t[:, k, blk],
                                                 start=(k == 0),
                                                 stop=(k == CK - 1))
                            for k in range(CK):
                                nc.tensor.matmul(sq_ps[:], ones_b[:],
                                                 sq_tiles[k][:],
                                                 start=(k == 0),
                                                 stop=(k == CK - 1))
                            mu = xbp.tile([128, 512], F32, name="muj", bufs=2)
                            nc.vector.tensor_scalar_mul(mu[:], mean_ps[:],
                                                        1.0 / C)
                            musq = xbp.tile([128, 512], F32, name="musqj",
                                            bufs=2)
                            nc.vector.tensor_mul(musq[:], mu[:], mu[:])
                            var = xbp.tile([128, 512], F32, name="varj",
                                           bufs=2)
                            nc.vector.scalar_tensor_tensor(
                                var[:], sq_ps[:], 1.0 / C, musq[:],
                                OP.mult, OP.subtract)
                            lnv = xbp.tile([128, 512], F32, name="lnvj",
                                           bufs=2)
                            nc.scalar.activation(lnv[:], var[:], AF.Ln,
                                                 bias=eps_t[:])
                            rb = xbp.tile([128, 512], BF16, name=f"rb{j}")
                            nc.scalar.activation(rb[:], lnv[:], AF.Exp,
                                                 scale=-0.5)
                            mbr = xbp.tile([1, 512], BF16, name=f"mbr{j}")
                            nc.vector.tensor_copy(mbr[:], mu[0:1, :])
                            rb_t.append(rb)
                            mbr_t.append(mbr)

                        # qkv = rstd*(x @ W' - mu*colsum(W'))
                        for jh in range(2):
                            js = [4 * jh + i for i in range(4)]
                            for m in range(3):
                                msl = slice(128 * m, 128 * (m + 1))
                                pms = []
                                for j in js:
                                    pm = qps.tile([128, 512], F32,
                                                  name="qkvp", bufs=2)
                                    nc.tensor.matmul(
                                        pm[:], wqsn_t[:, msl], mbr_t[j][:],
                                        start=True, stop=False)
                                    pms.append(pm)
                                for k in range(CK):
                                    for ji, j in enumerate(js):
                                        nc.tensor.matmul(
                                            pms[ji][:], wq_t[:, k, msl],
                                            xb_t[:, k, 512 * j:512 * (j + 1)],
                                            start=False,
                                            stop=(k == CK - 1))
                                for ji, j in enumerate(js):
                                    blk = slice(512 * j, 512 * (j + 1))
                                    if m == 0:
                                        nc.vector.tensor_mul(
                                            qp0[0:64, blk], pms[ji][0:64, :],
                                            rb_t[j][0:64, :])
                                        nc.vector.tensor_mul(
                                            qp1[64:128, blk],
                                            pms[ji][64:128, :],
                                            rb_t[j][64:128, :])
                                    else:
                                        nc.vector.tensor_mul(
                                            qdst[m][:, blk], pms[ji][:],
                                            rb_t[j][:])

                    # v_ext: transpose vT into per-head [kpos,64]+ones tiles
                    ve = [[], []]
                    with tc.tile_pool(name="veps", bufs=1, space="PSUM") as vps:
                        for t in range(T // 128):
                            tp = vps.tile([128, 128], BF16, name="vtp", bufs=2)
                            nc.tensor.transpose(tp[:],
                                                vT_t[:, 128 * t:128 * (t + 1)],
                                                ident_t[:])
                            for h in range(2):
                                vx = ap.tile([128, 65], BF16, name=f"ve{h}_{t}")
                                nc.vector.tensor_copy(
                                    vx[:, 0:64], tp[:, 64 * h:64 * (h + 1)])
                                nc.vector.memset(vx[:, 64:65], 1.0)
                                ve[h].append(vx)

                    # flash attention (no max subtraction), 2 heads
                    # interleaved per k-tile: both heads' scores for one
                    # k-tile share the stationary and one batched exp.
                    # A2A staging stores stream out per finished q-tile.
                    qp = [qp0, qp1]
                    oTe = [oTe0, oTe1]
                    with tc.tile_pool(name="atw", bufs=1) as aw, \
                         tc.tile_pool(name="atps", bufs=1, space="PSUM") as aps, \
                         tc.tile_pool(name="atpo", bufs=1, space="PSUM") as apo:
                        for qi in range(NCORES):
                            nkb = 4 * (qi + 1)
                            qsl = [qp[h][:, 512 * qi:512 * (qi + 1)]
                                   for h in range(2)]
                            opsb = [apo.tile([65, 512], F32, name=f"ops{h}",
                                             bufs=1) for h in range(2)]
                            for kb in range(nkb):
                                sp = aps.tile([128, 2, 512], F32,
                                              name="sp", bufs=3)
                                for h in range(2):
                                    nc.tensor.matmul(
                                        sp[:, h, :],
                                        kT_t[:, 128 * kb:128 * (kb + 1)],
                                        qsl[h], start=True, stop=True)
                                est = aw.tile([128, 2, 512], BF16,
                                              name="est", bufs=4)
                                nc.scalar.activation(est[:], sp[:],
                                                     AF.Exp, scale=0.125)
                                if kb >= 4 * qi:  # diagonal: causal mask
                                    jm = kb - 4 * qi
                                    for h in range(2):
                                        nc.vector.tensor_mul(
                                            est[:, h, :], est[:, h, :],
                                            masks_t[:, 512 * jm:512 * (jm + 1)])
                                for h in range(2):
                                    nc.tensor.matmul(
                                        opsb[h][:], ve[h][kb][:],
                                        est[:, h, :],
                                        start=(kb == 0),
                                        stop=(kb == nkb - 1))
                            blk = slice(512 * qi, 512 * (qi + 1))
                            base = 130 * qi
                            for h in range(2):
                                nc.vector.tensor_copy(oTe[h][:, blk],
                                                      opsb[h][:])
                            # stream this q-block into the A2A staging buffer
                            nc.sync.dma_start(ob[base:base + 64, :],
                                              oTe0[0:64, blk])
                            nc.sync.dma_start(ob[base + 64:base + 128, :],
                                              oTe1[0:64, blk])
                            nc.sync.dma_start(ob[base + 128:base + 129, :],
                                              oTe0[64:65, blk])
                            nc.sync.dma_start(ob[base + 129:base + 130, :],
                                              oTe1[64:65, blk])

                    if DEBUG:
                        nc.sync.dma_start(dbg["d_qp0"][:], qp0[:])
                        nc.sync.dma_start(dbg["d_qp1"][:], qp1[:])
                        nc.sync.dma_start(dbg["d_kT"][:], kT_t[:])
                        nc.sync.dma_start(dbg["d_vT"][:], vT_t[:])
                        nc.sync.dma_start(dbg["d_oTe0"][:], oTe0[:])
                        nc.sync.dma_start(dbg["d_oTe1"][:], oTe1[:])

                    # exchange head outputs + l rows: AllToAll
                    nc.gpsimd.collective_compute(
                        "AllToAll", OP.bypass, replica_groups=rg,
                        ins=[ob.opt()], outs=[oax.opt()])

                    # proj: x2T = xT + wproj.T @ (aoutT * 1/l)
                    oax_r = oax.rearrange("(k r) c -> r k c", r=130)
                    with tc.tile_pool(name="prs", bufs=1) as prs, \
                         tc.tile_pool(name="prps", bufs=1, space="PSUM") as pps:
                        wp_t = prs.tile([128, CK, C], BF16, name="wp_t")
                        nc.sync.dma_start(wp_t[:], wproj[:])
                        # all 16 heads' l for own columns -> one 1/l pass
                        l_all = prs.tile([16, 512], BF16, name="l_all")
                        for k in range(CK):
                            nc.sync.dma_start(
                                l_all[2 * k:2 * k + 2, :],
                                oax[130 * k + 128:130 * k + 130, :])
                        lln = prs.tile([16, 512], F32, name="lln")
                        nc.scalar.activation(lln[:], l_all[:], AF.Ln)
                        rl_all = prs.tile([16, 512], BF16, name="rl_all")
                        nc.scalar.activation(rl_all[:], lln[:], AF.Exp,
                                             scale=-1.0)
                        au_all = prs.tile([128, CK, 512], BF16, name="au_all")
                        nc.sync.dma_start(au_all[:], oax_r[0:128])
                        at_tiles = []
                        for k in range(CK):
                            rlb = pps.tile([128, 512], F32, name="rlb", bufs=2)
                            nc.tensor.matmul(
                                rlb[:], sel16_t[:, 128 * k:128 * (k + 1)],
                                rl_all[:], start=True, stop=True)
                            at = prs.tile([128, 512], BF16, name=f"at{k}")
                            nc.vector.tensor_mul(at[:], au_all[:, k, :],
                                                 rlb[:])
                            at_tiles.append(at)
                        for m in range(CK):
                            x2ps = pps.tile([128, 512], F32, name="x2p",
                                            bufs=2)
                            for k in range(CK):
                                nc.tensor.matmul(
                                    x2ps[:], wp_t[:, k, 128 * m:128 * (m + 1)],
                                    at_tiles[k][:], start=(k == 0),
                                    stop=(k == CK - 1))
                            nc.vector.tensor_add(
                                x2T_t[:, m, :], x2ps[:],
                                xT_t[:, m, :].bitcast(F32))

                if DEBUG:
                    nc.sync.dma_start(dbg["d_x2T"][:], x2T_t[:].bitcast(F32))

                # ---------------- MLP scope ----------------
                with tc.tile_pool(name="mlp", bufs=1) as mp:
                    h2T_t = mp.tile([128, CK, 512], BF16, name="h2T_t")
                    _layer_norm(nc, tc, x2T_t, ln2w_t, h2T_t, eps_t,
                                ones_t, ones_b, "2")
                    gel = []
                    with tc.tile_pool(name="fcs", bufs=1) as fs, \
                         tc.tile_pool(name="fcps", bufs=1, space="PSUM") as fps:
                        for g in range(FC // 512):  # 8 groups of 4 m-blocks
                            wg = fs.tile([128, 4, CK, 128], BF16, name="wfcg",
                                         bufs=2)
                            nc.sync.dma_start(wg[:], wfc[:, 4 * g:4 * g + 4])
                            pf = fps.tile([128, 4, 512], F32, name="fcp",
                                          bufs=2)
                            for mm in range(4):
                                for k in range(CK):
                                    nc.tensor.matmul(
                                        pf[:, mm, :], wg[:, mm, k, :],
                                        h2T_t[:, k, :],
                                        start=(k == 0), stop=(k == CK - 1))
                            gl = mp.tile([128, 4, 512], BF16, name=f"gel{g}")
                            nc.scalar.activation(gl[:], pf[:], AF.Gelu)
                            gel.append(gl)
                    # second matmul: single pass, 8 psum accumulators
                    with tc.tile_pool(name="m2s", bufs=1) as m2s, \
                         tc.tile_pool(name="m2ps", bufs=1, space="PSUM") as m2ps:
                        x3ps = [m2ps.tile([128, 512], F32, name=f"x3p{i}")
                                for i in range(CK)]
                        for f4 in range(FC // 512):
                            wm = m2s.tile([128, 4, C], BF16, name="wm",
                                          bufs=2)
                            nc.sync.dma_start(wm[:],
                                              wmlp[:, 4 * f4:4 * f4 + 4, :])
                            for ff in range(4):
                                f = 4 * f4 + ff
                                for i in range(CK):
                                    nc.tensor.matmul(
                                        x3ps[i][:],
                                        wm[:, ff, 128 * i:128 * (i + 1)],
                                        gel[f // 4][:, f % 4, :],
                                        start=(f == 0),
                                        stop=(f == FC // 128 - 1))
                        for i in range(CK):
                            o32 = m2s.tile([128, 512], F32, name="o32",
                                           bufs=2)
                            nc.vector.tensor_add(
                                o32[:], x3ps[i][:],
                                x2T_t[:, i, :].bitcast(F32))
                            nc.sync.dma_start(
                                outT[128 * i:128 * (i + 1), :], o32[:])

    nc.compile()
    return nc


def _host_inputs(x, w_qkv, w_attn_proj, w_fc, w_mlp_proj, ln1_w, ln2_w):
    """Build the 8 per-core input maps."""
    bf = ml_dtypes.bfloat16
    x2 = np.ascontiguousarray(np.asarray(x, np.float32).reshape(T, C))
    w_qkv = np.asarray(w_qkv, np.float32)
    ln1_w = np.asarray(ln1_w, np.float32)
    masks = np.zeros((128, 4 * 512), np.float32)
    kk = np.arange(128)[:, None]
    qq = np.arange(512)[None, :]
    for j in range(4):
        masks[:, 512 * j:512 * (j + 1)] = (qq >= kk + 128 * j)
    masks = masks.astype(bf)
    ident = np.eye(128, dtype=np.float32).astype(bf)
    onesw = np.ones((128, 128), np.float32)
    # sel16[:, 128k + d] = 1 where row r == 2k + d//64 (head of dim d in
    # aout chunk k); broadcasts rl_all rows onto the head-dim rows.
    sel16 = np.zeros((16, CK * 128), np.float32)
    for k in range(CK):
        sel16[2 * k, 128 * k:128 * k + 64] = 1.0
        sel16[2 * k + 1, 128 * k + 64:128 * (k + 1)] = 1.0
    sel16 = sel16.astype(bf)
    ln2 = np.ascontiguousarray(np.asarray(ln2_w, np.float32).reshape(CK, 128).T)
    # full x, transposed + C-chunked, bf16: xb[p, k, t] = x[t, 128k+p]
    xball = np.ascontiguousarray(
        x2.T.reshape(CK, 128, T).transpose(1, 0, 2).astype(bf))
    wproj = np.asarray(w_attn_proj, np.float32).reshape(CK, 128, C) \
        .transpose(1, 0, 2).astype(bf)
    wfc = np.asarray(w_fc, np.float32).reshape(CK, 128, FC // 128, 128) \
        .transpose(1, 2, 0, 3).astype(bf)
    wmlp = np.asarray(w_mlp_proj, np.float32).reshape(FC // 128, 128, C) \
        .transpose(1, 0, 2).astype(bf)
    common = {
        "xb": xball,
        "wproj": np.ascontiguousarray(wproj),
        "wfc": np.ascontiguousarray(wfc),
        "wmlp": np.ascontiguousarray(wmlp),
        "ln2w": ln2, "masks": masks, "ident": ident,
        "onesw": onesw, "sel16": sel16,
        "zeros": np.zeros((64, T), bf),
    }
    in_maps = []
    for c in range(NCORES):
        xTc = np.ascontiguousarray(x2[TC * c:TC * (c + 1), :].T)
        wq = np.concatenate(
            [w_qkv[:, C * s + 128 * c:C * s + 128 * (c + 1)] for s in range(3)],
            axis=1)  # [C, 384] pre-folded with ln1 weight
        wq = wq * ln1_w[:, None]
        wqs = wq.sum(axis=0)  # [384]
        wqsn = np.ascontiguousarray((-wqs).reshape(1, 3 * 128).astype(bf))
        wq = np.ascontiguousarray(
            wq.reshape(CK, 128, 3 * 128).transpose(1, 0, 2).astype(bf))
        in_maps.append({"xT": xTc, "wqkv": wq, "wqsn": wqsn, **common})
    return in_maps


def _run(in_maps, **kw):
    key = ("nc", DEBUG)
    if key not in _CACHE:
        _CACHE[key] = _build()
    return bass_utils.run_bass_kernel_spmd(
        _CACHE[key], in_maps, core_ids=list(range(NCORES)), **kw)


def kernel(x, w_qkv, w_attn_proj, w_fc, w_mlp_proj, ln1_w, ln2_w):
    in_maps = _host_inputs(x, w_qkv, w_attn_proj, w_fc, w_mlp_proj,
                           ln1_w, ln2_w)
    res = _run(in_maps)
    out = np.empty((1, T, C), np.float32)
    for c in range(NCORES):
        out[0, TC * c:TC * (c + 1), :] = res.results[c]["outT"].T
    return out
